# revision 30
# baseline (speedup 1.0000x reference)
"""Single-head causal attention (B=4, T=4096, E=1024, H=128) on 8 trn2 cores.

Sharding (key-split): core c -> (batch b = c//2, piece p = c%2). The two
cores of a batch split the KEYS: within every 256-token block, piece p owns
the 128 tokens at offset 128p. Each core:
  - projects K/V only for its OWN 2048 keys (no duplicate K/V compute),
  - projects Q for ALL 4096 queries,
  - computes the partial softmax numerator o = sum_own exp(s) v and
    denominator l = sum_own exp(s) over its own keys only.
The host merges: out = (o0 + o1) / (l0 + l1) per batch. This trades a
duplicated Q projection (cheap) for the K/V projection duplication
(expensive) of a query-split, cutting tensor-engine work ~9%.

SPMD trick: xT arrives column-PERMUTED per core (own 128 first within each
256-block), so "own keys" sit at fixed in-tile offsets and the device
program is identical on all cores; the causal boundary mask strip is
per-core data. Queries stay in permuted order end-to-end; the host
unpermutes when scattering (the permutation is an involution).

Device algorithm (per core, transposed layouts):
  per round tt = 0..7 (q-tile = permuted query cols 512tt..+511):
    project (during attention of round tt-1):
      QT tile  = Wq @ x^T          [H, 512]  (contiguous cols)
      KT own   = Wk @ x_own^T      [H, 2, 128] -> kt blocks 2tt, 2tt+1
      VT own   = Wv @ x_own^T -> f16 -> PE-transpose -> V blocks [128t,128h]
    attention over own-key PAIRS g=0..tt (pair g = own blocks 2g, 2g+1):
      ST[i] = KT_blk^T @ QT_tile  [128k, 512q] into a 2-bank psum slab
      diagonal pair (g==tt) gets additive mask strips (per-core data)
      ONE exp over the slab  [128, 1024] -> PT f16
      for i: OT += V_blk^T @ PT[i]; pacc[i] += PT[i]  (DVE)
    lb = allones^T @ (pacc0+pacc1); ship raw OT and lb row (no normalize)
"""

import numpy as np

B, T, E, H = 4, 4096, 1024, 128
P = 128
NB_E = E // P           # 8 contraction chunks
N_RND = 8               # rounds; round tt = permuted query cols 512tt..+511
SCALE = float(H) ** -0.5
NEG = -30000.0
N_CORES = 8
F32 = np.float32


def _perm_cols(p: int) -> np.ndarray:
    """Permuted token order for core piece p: within each 256-token block the
    own 128 tokens (offset 128p) come first, the other 128 after."""
    out = []
    for g in range(16):
        own = np.arange(256 * g + 128 * p, 256 * g + 128 * p + 128)
        oth = np.arange(256 * g + 128 * (1 - p), 256 * g + 128 * (1 - p) + 128)
        out.append(own)
        out.append(oth)
    return np.concatenate(out)


def _mask_pair(p: int) -> np.ndarray:
    """maskP [128, 2, 512] f16 for the diagonal pair of any round.

    Partition kk = key within own block i (i=0,1); col r = permuted in-tile
    query. Own block i holds absolute keys (512tt +) 256i + 128p + kk; the
    permuted query col r is absolute offset off(r) = 256*(r//256) +
    (128p if (r//128)%2==0 else 128*(1-p)) + r%128. Visible iff
    off(r) >= 256i + 128p + kk.
    """
    kk = np.arange(128)[:, None, None]
    i = np.arange(2)[None, :, None]
    r = np.arange(512)[None, None, :]
    s = r // 128
    own = (s % 2) == 0
    off = 256 * (r // 256) + np.where(own, 128 * p, 128 * (1 - p)) + r % 128
    visible = off >= 256 * i + 128 * p + kk
    return np.where(visible, 0.0, NEG).astype(np.float16)


def _emit(tc, aps):
    from concourse import mybir

    nc = tc.nc
    f32 = mybir.dt.float32
    f16 = mybir.dt.float16
    EXP = mybir.ActivationFunctionType.Exp

    from contextlib import ExitStack

    xS, wS, maskP, out_o, out_l = aps

    ctx = ExitStack()
    with ctx:
        # ---- pools ----
        consts = ctx.enter_context(tc.tile_pool(name="consts", bufs=1))
        x_pool = ctx.enter_context(tc.tile_pool(name="x", bufs=3))
        qt_pool = ctx.enter_context(tc.tile_pool(name="qt", bufs=2))
        vt_pool = ctx.enter_context(tc.tile_pool(name="vt", bufs=2))
        pt_pool = ctx.enter_context(tc.tile_pool(name="pt", bufs=3))
        pa_pool = ctx.enter_context(tc.tile_pool(name="pa", bufs=4))
        osb_pool = ctx.enter_context(tc.tile_pool(name="osb", bufs=2))
        # PSUM: score pair slabs 2x2 banks + proj 2 + ot 2 = 8 banks.
        # lb squats in an idle s_ps slot (scores are done when lb runs).
        s_ps = ctx.enter_context(tc.tile_pool(name="sps", bufs=2, space="PSUM"))
        o_ps = ctx.enter_context(tc.tile_pool(name="ops", bufs=2, space="PSUM"))
        p_ps = ctx.enter_context(tc.tile_pool(name="pps", bufs=2, space="PSUM"))

        # ---- persistent SBUF tensors ----
        allones = consts.tile([P, P], f16)
        identity = consts.tile([P, P], f16)
        w_sb = consts.tile([P, 3, NB_E, P], f16)   # [., (k|v|q), chunk, .]
        mask_sb = consts.tile([P, 2, 512], f16)
        kt_all = consts.tile([P, 16, P], f16)
        v_all = consts.tile([P, 16, P], f16)
        l_row = consts.tile([1, T], f32)

        # ---- x round tiles: [128, chunk c, block b(128), 128] f16 ----
        # own key tokens of the round sit at b = 0 and 2.
        x_tiles = [x_pool.tile([P, NB_E, 4, 128], f16, tag="x", name=f"x_{tt}")
                   for tt in range(N_RND)]

        def dma_x(tt, split=1, eng=None):
            """Load round tt's 512 permuted query cols. The host pre-swizzles
            xS to [tt, p, c, b, q], so every piece is a dense 2D transfer
            with multi-KB contiguous lines per partition."""
            engs = eng if eng is not None else [nc.sync]
            cw = NB_E // split
            for g in range(split):
                engs[g % len(engs)].dma_start(
                    x_tiles[tt][:, g * cw:(g + 1) * cw, :, :],
                    xS[tt, :, g * cw:(g + 1) * cw, :, :],
                )

        nc.gpsimd.memset(allones[:], 1.0)
        from concourse.masks import make_identity
        make_identity(nc, identity[:])

        # startup-critical order: weights first on sync (they gate every
        # projection matmul), x0 fine-grained across the scalar+gpsimd DMA
        # channels (per-channel bandwidth is the limit), mask + later
        # rounds behind them on sync.
        # weights slot-major, K first: the first projection matmul only
        # waits for the 256KB K slice instead of the full 768KB
        for sl_w in range(3):
            nc.sync.dma_start(w_sb[:, sl_w, :, :], wS[:, sl_w, :, :])
        dma_x(0, split=4, eng=[nc.scalar, nc.gpsimd])
        nc.sync.dma_start(mask_sb[:, :, :], maskP[:, :, :])
        dma_x(1, split=2, eng=[nc.scalar, nc.gpsimd])
        dma_x(2, split=2, eng=[nc.scalar, nc.gpsimd])

        # PE warmup: dummy matmuls ramp the tensor-engine p-state while the
        # first DMAs stream; their results are never read. 40 x ~107ns cold
        # = 4.3us of CONTINUOUS busy -- past the 3.4us HAM window, so the
        # clock reaches 2.4GHz during warmup (30 x 107 = 3.2us was just
        # UNDER the window and the clock stayed cold through round 0).
        for _ in range(40):
            wp = p_ps.tile([P, P], f32, tag="pps", name="warm")
            nc.tensor.matmul(wp[:], lhsT=allones[:], rhs=allones[:],
                             start=True, stop=True)

        # ---- projection pieces (generators of thunks) ----
        def q_group(tt, qt):
            xt = x_tiles[tt]
            ps = p_ps.tile([P, 512], f32, tag="pps")
            for c in range(NB_E):
                def mm(c=c, ps=ps):
                    nc.tensor.matmul(ps[:], lhsT=w_sb[:, 2, c, :],
                                     rhs=xt[:, c, :, :], start=(c == 0),
                                     stop=(c == NB_E - 1))
                yield mm
            # Q fin on ACT: keeps the DVE queue (mask/pacc critical ops) short
            yield lambda ps=ps: nc.scalar.copy(qt[:], ps[:])

        def k_group(tt):
            xt = x_tiles[tt]
            ps = p_ps.tile([P, 256], f32, tag="pps", name=f"kps_{tt}")
            for c in range(NB_E):
                def mm(c=c, ps=ps):
                    nc.tensor.matmul(ps[:], lhsT=w_sb[:, 0, c, :],
                                     rhs=xt[:, c, 0::2, :], start=(c == 0),
                                     stop=(c == NB_E - 1))
                yield mm

            def fin(ps=ps):
                nc.vector.tensor_copy(
                    kt_all[:, 2 * tt:2 * tt + 2, :],
                    ps[:].rearrange("p (i q) -> p i q", i=2))
            yield fin

        def v_group(tt):
            xt = x_tiles[tt]
            vt = vt_pool.tile([P, 256], f16, tag="vt", name=f"vt_{tt}")
            ps = p_ps.tile([P, 256], f32, tag="pps", name=f"vps_{tt}")
            for c in range(NB_E):
                def mm(c=c, ps=ps):
                    nc.tensor.matmul(ps[:], lhsT=w_sb[:, 1, c, :],
                                     rhs=xt[:, c, 0::2, :], start=(c == 0),
                                     stop=(c == NB_E - 1))
                yield mm
            yield lambda vt=vt, ps=ps: nc.vector.tensor_copy(vt[:], ps[:])
            for u in range(2):
                kb = 2 * tt + u

                def tr(u=u, kb=kb, vt=vt):
                    tp = p_ps.tile([P, P], f16, tag="pps", name=f"tp_{kb}")
                    nc.tensor.transpose(tp[:], vt[:, u * P:(u + 1) * P],
                                        identity[:])
                    nc.vector.tensor_copy(v_all[:, kb, :], tp[:])
                yield tr

        def chain(*gens):
            for g in gens:
                yield from g

        def drain(gen, n):
            if gen is None:
                return False
            for _ in range(n):
                try:
                    next(gen)()
                except StopIteration:
                    return False
            return True

        qts = [qt_pool.tile([P, 512], f16, tag="qt", name=f"qt_{t}")
               for t in range(N_RND)]

        # round 0 projections run up front. K first: its first matmul only
        # needs the first x chunk-pair off the wire; Q (which needs all of
        # x round 0) goes last.
        for piece in chain(k_group(0), v_group(0), q_group(0, qts[0])):
            piece()

        N_GEN = 9 + 9 + 11   # q + k + v piece counts per round
        gen_box = [None]

        def drain_gen(n):
            if gen_box[0] is not None and not drain(gen_box[0], n):
                gen_box[0] = None

        def new_round(tt):
            ot = o_ps.tile([P, 512], f32, tag="ops", name=f"ot_{tt}")
            pacc = pa_pool.tile([P, 2, 512], f16, tag="pa", name=f"pa_{tt}")
            nc.gpsimd.memset(pacc[:], 0.0)
            return {"tt": tt, "qs": qts[tt], "ot": ot, "pacc": pacc}

        s_tiles = {}

        def emit_scores(st, g):
            """Score pair g: own blocks 2g, 2g+1 into one 2-bank slab."""
            tt, qs = st["tt"], st["qs"]
            s = s_ps.tile([P, 2, 512], f32, tag="sps", name=f"s_{tt}_{g}")
            for i in range(2):
                c0 = 256 * i if g == tt else 0
                nc.tensor.matmul(
                    s[:, i, c0:512],
                    lhsT=kt_all[:, 2 * g + i, :],
                    rhs=qs[:, c0:512],
                    start=True, stop=True,
                )
            s_tiles[(tt, g)] = s

        def do_pair(st, g, first, last, rate):
            tt, ot, pacc = st["tt"], st["ot"], st["pacc"]
            s = s_tiles.pop((tt, g))
            if g == tt:  # diagonal pair: causal boundary mask strips
                nc.vector.tensor_add(s[:, 0, 0:256], s[:, 0, 0:256],
                                     mask_sb[:, 0, 0:256])
                nc.vector.tensor_add(s[:, 1, 256:512], s[:, 1, 256:512],
                                     mask_sb[:, 1, 256:512])
            pt = pt_pool.tile([P, 2, 512], f16, tag="pt")
            if g == tt:
                # diagonal pair: slot 1 cols [0,256) were never written
                # (causally dead); exp each slot's live range separately
                nc.scalar.activation(pt[:, 0, :], s[:, 0, :],
                                     EXP, scale=SCALE)
                nc.scalar.activation(pt[:, 1, 256:512], s[:, 1, 256:512],
                                     EXP, scale=SCALE)
            else:
                nc.scalar.activation(
                    pt[:].rearrange("p i q -> p (i q)"),
                    s[:].rearrange("p i q -> p (i q)"),
                    EXP, scale=SCALE)

            # projection pieces go to the PE queue HERE, between the
            # score matmuls and the PV matmuls: the in-order PE works
            # through them while the exp latency drains, instead of
            # head-blocking on PV.
            drain_gen(rate)

            for i in range(2):
                c0 = 256 * i if g == tt else 0
                nc.tensor.matmul(
                    ot[:, c0:512],
                    lhsT=v_all[:, 2 * g + i, :],
                    rhs=pt[:, i, c0:512],
                    start=(first and i == 0),
                    stop=(last and i == 1),
                )
            if g == tt:
                # diagonal: trim per slot (slot 1 cols [0,256) hold
                # exp(garbage) from the untouched psum region)
                nc.vector.tensor_add(pacc[:, 0, 0:512], pacc[:, 0, 0:512],
                                     pt[:, 0, 0:512])
                nc.vector.tensor_add(pacc[:, 1, 256:512],
                                     pacc[:, 1, 256:512],
                                     pt[:, 1, 256:512])
            else:
                # one fused 3D add for the whole pair
                nc.vector.tensor_add(pacc[:], pacc[:], pt[:])

        def epilogue(st, nq):
            """Denominator + ship RAW numerator/denominator (host merges the
            two key-halves). lb lives in a projection-psum slot so the score
            slab rotation keeps its lookahead parity across rounds. The
            out DMAs stripe across queues: serializing them on one engine
            (~0.7us each) would extend the kernel tail."""
            tt, ot, pacc = st["tt"], st["ot"], st["pacc"]
            dma_engs = [nc.gpsimd, nc.sync, nc.scalar]
            lb = p_ps.tile([P, 512], f32, tag="pps", name=f"lb_{tt}")
            width = 512 // nq
            for qb in range(nq):          # all denominator matmuls first,
                hb = qb * width           # back-to-back on the PE
                sl = slice(hb, hb + width)
                nc.tensor.matmul(lb[:, sl], lhsT=allones[:],
                                 rhs=pacc[:, 0, sl], start=True, stop=False)
                nc.tensor.matmul(lb[:, sl], lhsT=allones[:],
                                 rhs=pacc[:, 1, sl], start=False, stop=True)
            for qb in range(nq):          # then the copy/DMA pipeline
                hb = qb * width
                sl = slice(hb, hb + width)
                o_sb = osb_pool.tile([P, width], f32, tag="osb",
                                     padded_shape=[P, 512])
                nc.vector.tensor_copy(o_sb[:], ot[:, sl])
                nc.vector.tensor_copy(l_row[0:1, tt * 512 + hb:
                                            tt * 512 + hb + width],
                                      lb[0:1, sl])
                dma_engs[qb % len(dma_engs)].dma_start(
                    out_o[:, tt * 512 + hb: tt * 512 + hb + width], o_sb[:])

        # ---- rounds 0..5: sequential, next round's projections interleave
        for tt in range(N_RND - 2):
            if 1 <= tt:
                # sync/gpsimd queues only: the scalar queue carries the exps
                # and a DMA descriptor issue there would stall the chain
                dma_x(tt + 2, split=2, eng=[nc.sync, nc.gpsimd])
            npair = tt + 1
            gen_box[0] = chain(q_group(tt + 1, qts[tt + 1]),
                               k_group(tt + 1), v_group(tt + 1))
            # leave ~8 pieces for the round BOUNDARY: the epilogue ->
            # diag-scores -> mask -> exp chain there has ~2us of latency the
            # PE would otherwise idle through (ceil-rates drained everything
            # mid-round and left the boundary empty)
            rate = max(2, (N_GEN - 8) // npair)
            st = new_round(tt)
            # diagonal FIRST: its K/V landed last round, and front-loading
            # its mask-add + split-exp latency keeps the round tail short
            order = [tt] + list(range(tt))
            emit_scores(st, order[0])
            for n, g in enumerate(order):
                if n + 1 < npair:
                    emit_scores(st, order[n + 1])
                do_pair(st, g, first=(n == 0), last=(n == npair - 1),
                        rate=rate)
            while drain(gen_box[0], 4):
                pass
            gen_box[0] = None
            epilogue(st, nq=1)

        # ---- rounds 6 and 7: software-pipelined into ONE pair stream so
        # the final round's exp latency hides behind real work (round 7 has
        # no successor projections; alone it starves the PE and re-throttles
        # the clock). Round 7's own q/k/v land early in the merged stream.
        st6 = new_round(N_RND - 2)
        st7 = new_round(N_RND - 1)
        gen_box[0] = chain(q_group(N_RND - 1, qts[N_RND - 1]),
                           k_group(N_RND - 1), v_group(N_RND - 1))
        seq = [(st6, 6), (st6, 0), (st6, 1), (st7, 0), (st6, 2), (st7, 1),
               (st6, 3), (st7, 2), (st6, 4), (st7, 3), (st6, 5), (st7, 4),
               (st7, 5), (st7, 7), (st7, 6)]
        # q (first 9 pieces) must land before pair (7,0)'s scores emit at
        # position 2; the rest spreads evenly so no stretch of the stream
        # runs without filler for the exp latency.
        rates = [5, 4, 2, 2, 2, 2, 2, 2, 2, 2, 1, 1, 1, 1, 0]
        firsts = {}
        lasts = {}
        for n, (st, g) in enumerate(seq):
            firsts.setdefault(id(st), n)
            lasts[id(st)] = n
        emit_scores(*seq[0])
        for n, (st, g) in enumerate(seq):
            if n + 1 < len(seq):
                emit_scores(*seq[n + 1])
            do_pair(st, g, first=(firsts[id(st)] == n),
                    last=(lasts[id(st)] == n), rate=rates[n])
            if n == lasts[id(st6)]:
                epilogue(st6, nq=1)
        while drain(gen_box[0], 4):
            pass
        epilogue(st7, nq=4)
        nc.sync.dma_start(out_l[:, :], l_row[0:1, :])


def build_program():
    import concourse.tile as tile
    from concourse import bacc, mybir

    f32 = mybir.dt.float32
    f16 = mybir.dt.float16
    nc = bacc.Bacc("TRN2", target_bir_lowering=False, debug=False,
                   num_devices=N_CORES)
    # xS: host-swizzled [round, partition, chunk, block, q] so DMA reads are
    # dense 2D with per-partition-contiguous multi-KB lines. wS likewise.
    xS = nc.dram_tensor("xS", [N_RND, P, NB_E, 4, 128], f16,
                        kind="ExternalInput").ap()
    wS = nc.dram_tensor("wS", [P, 3, NB_E, H], f16, kind="ExternalInput").ap()
    maskP = nc.dram_tensor("maskP", [128, 2, 512], f16,
                           kind="ExternalInput").ap()
    out_o = nc.dram_tensor("out_o", [H, T], f32, kind="ExternalOutput").ap()
    out_l = nc.dram_tensor("out_l", [1, T], f32, kind="ExternalOutput").ap()

    with tile.TileContext(nc) as tc:
        _emit(tc, (xS, wS, maskP, out_o, out_l))
    nc.compile()
    return nc


def make_in_maps(x, Wq, Wk, Wv):
    """Per-core input maps. x: [B,T,E] f32; W*: [H,E] f32."""
    x = np.asarray(x, dtype=F32)
    # wS[p, s, c, h] = W_s^T[c*128+p, h]  (slot-major, order k, v, q)
    wqkv = np.stack(
        [np.asarray(Wk, F32).T, np.asarray(Wv, F32).T, np.asarray(Wq, F32).T],
        axis=1).astype(np.float16)                      # [E, 3, H]
    wS = np.ascontiguousarray(
        wqkv.reshape(NB_E, P, 3, H).transpose(1, 2, 0, 3))
    masks = [_mask_pair(0), _mask_pair(1)]
    perms = [_perm_cols(0), _perm_cols(1)]
    in_maps = []
    for c in range(N_CORES):
        b, p = c // 2, c % 2
        xb = x[b][perms[p]]                                    # [T, E] permuted
        xT_np = xb.T.astype(np.float16)                        # [E, T]
        # xS[tt, p, c, bb, q] = xT[c*128+p, tt*512 + bb*128 + q]
        xS = np.ascontiguousarray(
            xT_np.reshape(NB_E, P, N_RND, 4, 128).transpose(2, 1, 0, 3, 4))
        in_maps.append({
            "xS": xS,
            "wS": wS,
            "maskP": masks[p],
        })
    return in_maps


def run(x, Wq, Wk, Wv, trace=False, trace_cores=None):
    """Returns (full_output [B,T,H] f32, BassKernelResults)."""
    from concourse.bass_utils import run_bass_kernel_spmd

    nc = build_program()
    in_maps = make_in_maps(x, Wq, Wk, Wv)
    res = run_bass_kernel_spmd(
        nc, in_maps, list(range(N_CORES)), trace=trace,
        trace_cores=trace_cores,
    )
    perms = [_perm_cols(0), _perm_cols(1)]
    full = np.empty((B, T, H), dtype=F32)
    for b in range(B):
        o_sum = np.zeros((H, T), dtype=np.float64)
        l_sum = np.zeros(T, dtype=np.float64)
        for p in range(2):
            r = res.results[2 * b + p]
            o_nat = np.empty((H, T), dtype=np.float64)
            l_nat = np.empty(T, dtype=np.float64)
            o_nat[:, perms[p]] = r["out_o"].astype(np.float64)
            l_nat[perms[p]] = r["out_l"][0].astype(np.float64)
            o_sum += o_nat
            l_sum += l_nat
        full[b] = (o_sum / l_sum).T.astype(F32)
    return full, res


def kernel(x, Wq, Wk, Wv):
    full, _ = run(x, Wq, Wk, Wv, trace=False)
    return full


if __name__ == "__main__":
    nc = build_program()
    print("program built ok")


# revision 31
# speedup vs baseline: 1.0011x; 1.0011x over previous
"""Single-head causal attention (B=4, T=4096, E=1024, H=128) on 8 trn2 cores.

Sharding (key-split): core c -> (batch b = c//2, piece p = c%2). The two
cores of a batch split the KEYS: within every 256-token block, piece p owns
the 128 tokens at offset 128p. Each core:
  - projects K/V only for its OWN 2048 keys (no duplicate K/V compute),
  - projects Q for ALL 4096 queries,
  - computes the partial softmax numerator o = sum_own exp(s) v and
    denominator l = sum_own exp(s) over its own keys only.
The host merges: out = (o0 + o1) / (l0 + l1) per batch. This trades a
duplicated Q projection (cheap) for the K/V projection duplication
(expensive) of a query-split, cutting tensor-engine work ~9%.

SPMD trick: xT arrives column-PERMUTED per core (own 128 first within each
256-block), so "own keys" sit at fixed in-tile offsets and the device
program is identical on all cores; the causal boundary mask strip is
per-core data. Queries stay in permuted order end-to-end; the host
unpermutes when scattering (the permutation is an involution).

Device algorithm (per core, transposed layouts):
  per round tt = 0..7 (q-tile = permuted query cols 512tt..+511):
    project (during attention of round tt-1):
      QT tile  = Wq @ x^T          [H, 512]  (contiguous cols)
      KT own   = Wk @ x_own^T      [H, 2, 128] -> kt blocks 2tt, 2tt+1
      VT own   = Wv @ x_own^T -> f16 -> PE-transpose -> V blocks [128t,128h]
    attention over own-key PAIRS g=0..tt (pair g = own blocks 2g, 2g+1):
      ST[i] = KT_blk^T @ QT_tile  [128k, 512q] into a 2-bank psum slab
      diagonal pair (g==tt) gets additive mask strips (per-core data)
      ONE exp over the slab  [128, 1024] -> PT f16
      for i: OT += V_blk^T @ PT[i]; pacc[i] += PT[i]  (DVE)
    lb = allones^T @ (pacc0+pacc1); ship raw OT and lb row (no normalize)
"""

import numpy as np

B, T, E, H = 4, 4096, 1024, 128
P = 128
NB_E = E // P           # 8 contraction chunks
N_RND = 8               # rounds; round tt = permuted query cols 512tt..+511
SCALE = float(H) ** -0.5
NEG = -30000.0
N_CORES = 8
F32 = np.float32


def _perm_cols(p: int) -> np.ndarray:
    """Permuted token order for core piece p: within each 256-token block the
    own 128 tokens (offset 128p) come first, the other 128 after."""
    out = []
    for g in range(16):
        own = np.arange(256 * g + 128 * p, 256 * g + 128 * p + 128)
        oth = np.arange(256 * g + 128 * (1 - p), 256 * g + 128 * (1 - p) + 128)
        out.append(own)
        out.append(oth)
    return np.concatenate(out)


def _mask_pair(p: int) -> np.ndarray:
    """maskP [128, 2, 512] f16 for the diagonal pair of any round.

    Partition kk = key within own block i (i=0,1); col r = permuted in-tile
    query. Own block i holds absolute keys (512tt +) 256i + 128p + kk; the
    permuted query col r is absolute offset off(r) = 256*(r//256) +
    (128p if (r//128)%2==0 else 128*(1-p)) + r%128. Visible iff
    off(r) >= 256i + 128p + kk.
    """
    kk = np.arange(128)[:, None, None]
    i = np.arange(2)[None, :, None]
    r = np.arange(512)[None, None, :]
    s = r // 128
    own = (s % 2) == 0
    off = 256 * (r // 256) + np.where(own, 128 * p, 128 * (1 - p)) + r % 128
    visible = off >= 256 * i + 128 * p + kk
    return np.where(visible, 0.0, NEG).astype(np.float16)


def _emit(tc, aps):
    from concourse import mybir

    nc = tc.nc
    f32 = mybir.dt.float32
    f16 = mybir.dt.float16
    EXP = mybir.ActivationFunctionType.Exp

    from contextlib import ExitStack

    xS, wS, maskP, out_o, out_l = aps

    ctx = ExitStack()
    with ctx:
        # ---- pools ----
        consts = ctx.enter_context(tc.tile_pool(name="consts", bufs=1))
        x_pool = ctx.enter_context(tc.tile_pool(name="x", bufs=3))
        qt_pool = ctx.enter_context(tc.tile_pool(name="qt", bufs=2))
        vt_pool = ctx.enter_context(tc.tile_pool(name="vt", bufs=2))
        pt_pool = ctx.enter_context(tc.tile_pool(name="pt", bufs=3))
        pa_pool = ctx.enter_context(tc.tile_pool(name="pa", bufs=4))
        osb_pool = ctx.enter_context(tc.tile_pool(name="osb", bufs=2))
        # PSUM: score pair slabs 2x2 banks + proj 2 + ot 2 = 8 banks.
        # lb squats in an idle s_ps slot (scores are done when lb runs).
        s_ps = ctx.enter_context(tc.tile_pool(name="sps", bufs=2, space="PSUM"))
        o_ps = ctx.enter_context(tc.tile_pool(name="ops", bufs=2, space="PSUM"))
        p_ps = ctx.enter_context(tc.tile_pool(name="pps", bufs=2, space="PSUM"))

        # ---- persistent SBUF tensors ----
        allones = consts.tile([P, P], f16)
        identity = consts.tile([P, P], f16)
        w_sb = consts.tile([P, 3, NB_E, P], f16)   # [., (k|v|q), chunk, .]
        mask_sb = consts.tile([P, 2, 512], f16)
        kt_all = consts.tile([P, 16, P], f16)
        v_all = consts.tile([P, 16, P], f16)
        l_row = consts.tile([1, T], f32)

        # ---- x round tiles: [128, chunk c, block b(128), 128] f16 ----
        # own key tokens of the round sit at b = 0 and 2.
        x_tiles = [x_pool.tile([P, NB_E, 4, 128], f16, tag="x", name=f"x_{tt}")
                   for tt in range(N_RND)]

        def dma_x(tt, split=1, eng=None):
            """Load round tt's 512 permuted query cols. The host pre-swizzles
            xS to [tt, p, c, b, q], so every piece is a dense 2D transfer
            with multi-KB contiguous lines per partition."""
            engs = eng if eng is not None else [nc.sync]
            cw = NB_E // split
            for g in range(split):
                engs[g % len(engs)].dma_start(
                    x_tiles[tt][:, g * cw:(g + 1) * cw, :, :],
                    xS[tt, :, g * cw:(g + 1) * cw, :, :],
                )

        nc.gpsimd.memset(allones[:], 1.0)
        from concourse.masks import make_identity
        make_identity(nc, identity[:])

        # startup-critical order: weights first on sync (they gate every
        # projection matmul), x0 fine-grained across the scalar+gpsimd DMA
        # channels (per-channel bandwidth is the limit), mask + later
        # rounds behind them on sync.
        # weights slot-major, K first: the first projection matmul only
        # waits for the 256KB K slice instead of the full 768KB
        for sl_w in range(3):
            nc.sync.dma_start(w_sb[:, sl_w, :, :], wS[:, sl_w, :, :])
        dma_x(0, split=4, eng=[nc.scalar, nc.gpsimd])
        nc.sync.dma_start(mask_sb[:, :, :], maskP[:, :, :])
        dma_x(1, split=2, eng=[nc.scalar, nc.gpsimd])
        dma_x(2, split=2, eng=[nc.scalar, nc.gpsimd])

        # PE warmup: dummy matmuls ramp the tensor-engine p-state while the
        # first DMAs stream; their results are never read. 40 x ~107ns cold
        # = 4.3us of CONTINUOUS busy -- past the 3.4us HAM window, so the
        # clock reaches 2.4GHz during warmup (30 x 107 = 3.2us was just
        # UNDER the window and the clock stayed cold through round 0).
        for _ in range(40):
            wp = p_ps.tile([P, P], f32, tag="pps", name="warm")
            nc.tensor.matmul(wp[:], lhsT=allones[:], rhs=allones[:],
                             start=True, stop=True)

        # ---- projection pieces (generators of thunks) ----
        def q_group(tt, qt):
            xt = x_tiles[tt]
            ps = p_ps.tile([P, 512], f32, tag="pps")
            for c in range(NB_E):
                def mm(c=c, ps=ps):
                    nc.tensor.matmul(ps[:], lhsT=w_sb[:, 2, c, :],
                                     rhs=xt[:, c, :, :], start=(c == 0),
                                     stop=(c == NB_E - 1))
                yield mm
            # Q fin on ACT: keeps the DVE queue (mask/pacc critical ops) short
            yield lambda ps=ps: nc.scalar.copy(qt[:], ps[:])

        def k_group(tt):
            xt = x_tiles[tt]
            ps = p_ps.tile([P, 256], f32, tag="pps", name=f"kps_{tt}")
            for c in range(NB_E):
                def mm(c=c, ps=ps):
                    nc.tensor.matmul(ps[:], lhsT=w_sb[:, 0, c, :],
                                     rhs=xt[:, c, 0::2, :], start=(c == 0),
                                     stop=(c == NB_E - 1))
                yield mm

            def fin(ps=ps):
                nc.vector.tensor_copy(
                    kt_all[:, 2 * tt:2 * tt + 2, :],
                    ps[:].rearrange("p (i q) -> p i q", i=2))
            yield fin

        def v_group(tt):
            xt = x_tiles[tt]
            vt = vt_pool.tile([P, 256], f16, tag="vt", name=f"vt_{tt}")
            ps = p_ps.tile([P, 256], f32, tag="pps", name=f"vps_{tt}")
            for c in range(NB_E):
                def mm(c=c, ps=ps):
                    nc.tensor.matmul(ps[:], lhsT=w_sb[:, 1, c, :],
                                     rhs=xt[:, c, 0::2, :], start=(c == 0),
                                     stop=(c == NB_E - 1))
                yield mm
            yield lambda vt=vt, ps=ps: nc.vector.tensor_copy(vt[:], ps[:])
            for u in range(2):
                kb = 2 * tt + u

                def tr(u=u, kb=kb, vt=vt):
                    tp = p_ps.tile([P, P], f16, tag="pps", name=f"tp_{kb}")
                    nc.tensor.transpose(tp[:], vt[:, u * P:(u + 1) * P],
                                        identity[:])
                    nc.vector.tensor_copy(v_all[:, kb, :], tp[:])
                yield tr

        def chain(*gens):
            for g in gens:
                yield from g

        def drain(gen, n):
            if gen is None:
                return False
            for _ in range(n):
                try:
                    next(gen)()
                except StopIteration:
                    return False
            return True

        qts = [qt_pool.tile([P, 512], f16, tag="qt", name=f"qt_{t}")
               for t in range(N_RND)]

        # round 0 projections run up front. K first: its first matmul only
        # needs the first x chunk-pair off the wire; Q (which needs all of
        # x round 0) goes last.
        for piece in chain(k_group(0), v_group(0), q_group(0, qts[0])):
            piece()

        N_GEN = 9 + 9 + 11   # q + k + v piece counts per round
        gen_box = [None]

        def drain_gen(n):
            if gen_box[0] is not None and not drain(gen_box[0], n):
                gen_box[0] = None

        def new_round(tt):
            ot = o_ps.tile([P, 512], f32, tag="ops", name=f"ot_{tt}")
            pacc = pa_pool.tile([P, 2, 512], f16, tag="pa", name=f"pa_{tt}")
            nc.gpsimd.memset(pacc[:], 0.0)
            return {"tt": tt, "qs": qts[tt], "ot": ot, "pacc": pacc}

        s_tiles = {}

        def emit_scores(st, g):
            """Score pair g: own blocks 2g, 2g+1 into one 2-bank slab."""
            tt, qs = st["tt"], st["qs"]
            s = s_ps.tile([P, 2, 512], f32, tag="sps", name=f"s_{tt}_{g}")
            for i in range(2):
                c0 = 256 * i if g == tt else 0
                nc.tensor.matmul(
                    s[:, i, c0:512],
                    lhsT=kt_all[:, 2 * g + i, :],
                    rhs=qs[:, c0:512],
                    start=True, stop=True,
                )
            s_tiles[(tt, g)] = s

        def do_pair(st, g, first, last, rate):
            tt, ot, pacc = st["tt"], st["ot"], st["pacc"]
            s = s_tiles.pop((tt, g))
            if g == tt:  # diagonal pair: causal boundary mask strips
                nc.vector.tensor_add(s[:, 0, 0:256], s[:, 0, 0:256],
                                     mask_sb[:, 0, 0:256])
                nc.vector.tensor_add(s[:, 1, 256:512], s[:, 1, 256:512],
                                     mask_sb[:, 1, 256:512])
            pt = pt_pool.tile([P, 2, 512], f16, tag="pt")
            if g == tt:
                # diagonal pair: slot 1 cols [0,256) were never written
                # (causally dead); exp each slot's live range separately
                nc.scalar.activation(pt[:, 0, :], s[:, 0, :],
                                     EXP, scale=SCALE)
                nc.scalar.activation(pt[:, 1, 256:512], s[:, 1, 256:512],
                                     EXP, scale=SCALE)
            else:
                nc.scalar.activation(
                    pt[:].rearrange("p i q -> p (i q)"),
                    s[:].rearrange("p i q -> p (i q)"),
                    EXP, scale=SCALE)

            # projection pieces go to the PE queue HERE, between the
            # score matmuls and the PV matmuls: the in-order PE works
            # through them while the exp latency drains, instead of
            # head-blocking on PV.
            drain_gen(rate)

            for i in range(2):
                c0 = 256 * i if g == tt else 0
                nc.tensor.matmul(
                    ot[:, c0:512],
                    lhsT=v_all[:, 2 * g + i, :],
                    rhs=pt[:, i, c0:512],
                    start=(first and i == 0),
                    stop=(last and i == 1),
                )
            if g == tt:
                # diagonal: trim per slot (slot 1 cols [0,256) hold
                # exp(garbage) from the untouched psum region)
                nc.vector.tensor_add(pacc[:, 0, 0:512], pacc[:, 0, 0:512],
                                     pt[:, 0, 0:512])
                nc.vector.tensor_add(pacc[:, 1, 256:512],
                                     pacc[:, 1, 256:512],
                                     pt[:, 1, 256:512])
            else:
                # one fused 3D add for the whole pair
                nc.vector.tensor_add(pacc[:], pacc[:], pt[:])

        def epilogue(st, nq):
            """Denominator + ship RAW numerator/denominator (host merges the
            two key-halves). lb lives in a projection-psum slot so the score
            slab rotation keeps its lookahead parity across rounds. The
            out DMAs stripe across queues: serializing them on one engine
            (~0.7us each) would extend the kernel tail."""
            tt, ot, pacc = st["tt"], st["ot"], st["pacc"]
            dma_engs = [nc.gpsimd, nc.sync, nc.scalar]
            lb = p_ps.tile([P, 512], f32, tag="pps", name=f"lb_{tt}")
            width = 512 // nq
            for qb in range(nq):
                hb = qb * width
                sl = slice(hb, hb + width)
                nc.tensor.matmul(lb[:, sl], lhsT=allones[:],
                                 rhs=pacc[:, 0, sl], start=True, stop=False)
                nc.tensor.matmul(lb[:, sl], lhsT=allones[:],
                                 rhs=pacc[:, 1, sl], start=False, stop=True)
                o_sb = osb_pool.tile([P, width], f32, tag="osb",
                                     padded_shape=[P, 512])
                nc.vector.tensor_copy(o_sb[:], ot[:, sl])
                nc.vector.tensor_copy(l_row[0:1, tt * 512 + hb:
                                            tt * 512 + hb + width],
                                      lb[0:1, sl])
                dma_engs[qb % len(dma_engs)].dma_start(
                    out_o[:, tt * 512 + hb: tt * 512 + hb + width], o_sb[:])

        # ---- rounds 0..5: sequential, next round's projections interleave
        for tt in range(N_RND - 2):
            if 1 <= tt:
                # sync/gpsimd queues only: the scalar queue carries the exps
                # and a DMA descriptor issue there would stall the chain
                dma_x(tt + 2, split=2, eng=[nc.sync, nc.gpsimd])
            npair = tt + 1
            gen_box[0] = chain(q_group(tt + 1, qts[tt + 1]),
                               k_group(tt + 1), v_group(tt + 1))
            rate = -(-N_GEN // npair)
            st = new_round(tt)
            # diagonal FIRST: its K/V landed last round, and front-loading
            # its mask-add + split-exp latency keeps the round tail short
            order = [tt] + list(range(tt))
            emit_scores(st, order[0])
            for n, g in enumerate(order):
                if n + 1 < npair:
                    emit_scores(st, order[n + 1])
                do_pair(st, g, first=(n == 0), last=(n == npair - 1),
                        rate=rate)
            while drain(gen_box[0], 4):
                pass
            gen_box[0] = None
            epilogue(st, nq=1)

        # ---- rounds 6 and 7: software-pipelined into ONE pair stream so
        # the final round's exp latency hides behind real work (round 7 has
        # no successor projections; alone it starves the PE and re-throttles
        # the clock). Round 7's own q/k/v land early in the merged stream.
        st6 = new_round(N_RND - 2)
        st7 = new_round(N_RND - 1)
        gen_box[0] = chain(q_group(N_RND - 1, qts[N_RND - 1]),
                           k_group(N_RND - 1), v_group(N_RND - 1))
        seq = [(st6, 6), (st6, 0), (st6, 1), (st7, 0), (st6, 2), (st7, 1),
               (st6, 3), (st7, 2), (st6, 4), (st7, 3), (st6, 5), (st7, 4),
               (st7, 5), (st7, 7), (st7, 6)]
        # q (first 9 pieces) must land before pair (7,0)'s scores emit at
        # position 2; the rest spreads evenly so no stretch of the stream
        # runs without filler for the exp latency.
        rates = [5, 4, 2, 2, 2, 2, 2, 2, 2, 2, 1, 1, 1, 1, 0]
        firsts = {}
        lasts = {}
        for n, (st, g) in enumerate(seq):
            firsts.setdefault(id(st), n)
            lasts[id(st)] = n
        emit_scores(*seq[0])
        for n, (st, g) in enumerate(seq):
            if n + 1 < len(seq):
                emit_scores(*seq[n + 1])
            do_pair(st, g, first=(firsts[id(st)] == n),
                    last=(lasts[id(st)] == n), rate=rates[n])
            if n == lasts[id(st6)]:
                epilogue(st6, nq=1)
        while drain(gen_box[0], 4):
            pass
        epilogue(st7, nq=4)
        nc.sync.dma_start(out_l[:, :], l_row[0:1, :])


def build_program():
    import concourse.tile as tile
    from concourse import bacc, mybir

    f32 = mybir.dt.float32
    f16 = mybir.dt.float16
    nc = bacc.Bacc("TRN2", target_bir_lowering=False, debug=False,
                   num_devices=N_CORES)
    # xS: host-swizzled [round, partition, chunk, block, q] so DMA reads are
    # dense 2D with per-partition-contiguous multi-KB lines. wS likewise.
    xS = nc.dram_tensor("xS", [N_RND, P, NB_E, 4, 128], f16,
                        kind="ExternalInput").ap()
    wS = nc.dram_tensor("wS", [P, 3, NB_E, H], f16, kind="ExternalInput").ap()
    maskP = nc.dram_tensor("maskP", [128, 2, 512], f16,
                           kind="ExternalInput").ap()
    out_o = nc.dram_tensor("out_o", [H, T], f32, kind="ExternalOutput").ap()
    out_l = nc.dram_tensor("out_l", [1, T], f32, kind="ExternalOutput").ap()

    with tile.TileContext(nc) as tc:
        _emit(tc, (xS, wS, maskP, out_o, out_l))
    nc.compile()
    return nc


def make_in_maps(x, Wq, Wk, Wv):
    """Per-core input maps. x: [B,T,E] f32; W*: [H,E] f32."""
    x = np.asarray(x, dtype=F32)
    # wS[p, s, c, h] = W_s^T[c*128+p, h]  (slot-major, order k, v, q)
    wqkv = np.stack(
        [np.asarray(Wk, F32).T, np.asarray(Wv, F32).T, np.asarray(Wq, F32).T],
        axis=1).astype(np.float16)                      # [E, 3, H]
    wS = np.ascontiguousarray(
        wqkv.reshape(NB_E, P, 3, H).transpose(1, 2, 0, 3))
    masks = [_mask_pair(0), _mask_pair(1)]
    perms = [_perm_cols(0), _perm_cols(1)]
    in_maps = []
    for c in range(N_CORES):
        b, p = c // 2, c % 2
        xb = x[b][perms[p]]                                    # [T, E] permuted
        xT_np = xb.T.astype(np.float16)                        # [E, T]
        # xS[tt, p, c, bb, q] = xT[c*128+p, tt*512 + bb*128 + q]
        xS = np.ascontiguousarray(
            xT_np.reshape(NB_E, P, N_RND, 4, 128).transpose(2, 1, 0, 3, 4))
        in_maps.append({
            "xS": xS,
            "wS": wS,
            "maskP": masks[p],
        })
    return in_maps


def run(x, Wq, Wk, Wv, trace=False, trace_cores=None):
    """Returns (full_output [B,T,H] f32, BassKernelResults)."""
    from concourse.bass_utils import run_bass_kernel_spmd

    nc = build_program()
    in_maps = make_in_maps(x, Wq, Wk, Wv)
    res = run_bass_kernel_spmd(
        nc, in_maps, list(range(N_CORES)), trace=trace,
        trace_cores=trace_cores,
    )
    perms = [_perm_cols(0), _perm_cols(1)]
    full = np.empty((B, T, H), dtype=F32)
    for b in range(B):
        o_sum = np.zeros((H, T), dtype=np.float64)
        l_sum = np.zeros(T, dtype=np.float64)
        for p in range(2):
            r = res.results[2 * b + p]
            o_nat = np.empty((H, T), dtype=np.float64)
            l_nat = np.empty(T, dtype=np.float64)
            o_nat[:, perms[p]] = r["out_o"].astype(np.float64)
            l_nat[perms[p]] = r["out_l"][0].astype(np.float64)
            o_sum += o_nat
            l_sum += l_nat
        full[b] = (o_sum / l_sum).T.astype(F32)
    return full, res


def kernel(x, Wq, Wk, Wv):
    full, _ = run(x, Wq, Wk, Wv, trace=False)
    return full


if __name__ == "__main__":
    nc = build_program()
    print("program built ok")
